# revision 51
# baseline (speedup 1.0000x reference)
"""Fused 3-layer MLP (Linear+GELU, Linear+GELU, Linear) for Trainium2.

Strategy: 8-way data parallel over the batch dimension (16384 -> 2048 rows
per NeuronCore), weights replicated.  All operands are laid out on the host
so that the contraction dimension lands on SBUF partitions for both matmul
operands -- activations flow feature-partitioned ("transposed", [feat, batch])
through all three layers, so no on-device transposes are needed:

    h1T = gelu(W1 @ xT + b1)    lhsT = W1.T (pre-transposed on host)
    h2T = gelu(W2 @ h1T + b2)   lhsT = W2.T
    oT  =      W3 @ h2T + b3    lhsT = W3.T

Each layer's bias-add (+ GELU) is a single ScalarE activation op reading the
PSUM accumulator and writing SBUF.  The batch is processed in 4 chunks of 512
(one PSUM bank of fp32 per output tile).

Variants (MLP_VARIANT env var, default bf16):
  bf16: matmul operands rounded to bf16 (fp32 PSUM accumulate); W1/W2/x
        SBUF-resident, W3 streamed.  ~413 us HW, ~4.2e-3 l2 relative error.
  f32r: fp32 data rounded to the 11-mantissa-bit float32r grid; full PE rate
        at moving dim >= 512, but W2/W3 must be streamed and the 4-byte
        weight loads pace slower.  ~465 us HW, ~2.7e-4 l2 relative error.
"""

import os

import numpy as np
import ml_dtypes

# Problem shapes (fixed by the problem statement).
B, D_IN, H1, H2, D_OUT = 16384, 512, 2048, 2048, 1024
N_CORES = 8
B_L = B // N_CORES  # 2048 batch rows per core
P = 128
NCK = 512  # batch chunk = matmul moving free dim = one fp32 PSUM bank

VARIANT = os.environ.get("MLP_VARIANT", "bf16")  # "bf16" | "f32r"

LAST_RESULT = None  # BassKernelResults of the most recent run (for test.py)

_nc_cache = {}


def _build_nc(variant, b_l=B_L, nck=None, _act_name="Gelu"):
    import concourse.mybir as mybir
    import concourse.tile as tile
    from concourse import bacc

    dt = mybir.dt
    act = mybir.ActivationFunctionType
    act_fn = getattr(act, _act_name)
    bf16 = variant == "bf16"
    # float32r: fp32 with the mantissa rounded to 11 bits (low 12 bits of the
    # word zero).  The host pre-rounds all matmul inputs to that grid, so the
    # tensors are declared float32r end-to-end and the PE runs its
    # full-rate single-pass fp32r mode.
    DDT = dt.bfloat16 if bf16 else dt.float32r  # storage dtype
    if nck is None:
        # One fp32 PSUM bank per matmul output; a matmul may not cross the
        # bank boundary, so 512 is the widest legal moving dim.
        nck = 512
    chunks = b_l // nck
    psum_bufs = 6
    K1T, H1T, H2T, DOT = D_IN // P, H1 // P, H2 // P, D_OUT // P

    # Bacc (not raw Bass): its compile() pipeline legalizes multi-wait
    # instructions into event-semaphore pairs -- walrus accepts at most one
    # sync wait per instruction.
    nc = bacc.Bacc("TRN2", target_bir_lowering=False, debug=False)

    xT = nc.dram_tensor("xT", [D_IN, b_l], DDT, kind="ExternalInput").ap()
    # W1 is packed per output-column block (like the streamed weights below)
    # so layer 1 of chunk 0 can start after a single 256 KB block lands
    # instead of after the whole W1 load.
    w1t = nc.dram_tensor(
        "w1t", [H1 // P, P, D_IN // P, P], DDT, kind="ExternalInput"
    ).ap()
    # W2 is packed per output-column block on the host:
    # w2p[mi, p, ko, m] = W2T[ko*P+p, mi*P+m], so each [P, H1T, P] block DMA
    # reads contiguously per partition at full line rate, and blocks arrive
    # in the order layer 2 consumes them.
    w2t = nc.dram_tensor(
        "w2t", [H2 // P, P, H1 // P, P], DDT, kind="ExternalInput"
    ).ap()
    # W3 is always streamed (packed) -- it doesn't fit in SBUF alongside the
    # wide-chunk activations, and the DMA engines have plenty of headroom.
    w3t = nc.dram_tensor(
        "w3t", [D_OUT // P, P, H2 // P, P], DDT, kind="ExternalInput"
    ).ap()
    # Biases are host-transposed to [P, n_tiles] so the load is a plain
    # contiguous copy (the naive "(o p) -> p o" view is a 4-byte-element
    # gather whose descriptor generation alone costs ~2 us).
    b1 = nc.dram_tensor("b1", [P, H1 // P], dt.float32, kind="ExternalInput").ap()
    b2 = nc.dram_tensor("b2", [P, H2 // P], dt.float32, kind="ExternalInput").ap()
    b3 = nc.dram_tensor("b3", [P, D_OUT // P], dt.float32, kind="ExternalInput").ap()
    outT = nc.dram_tensor("outT", [D_OUT, b_l], dt.float32, kind="ExternalOutput").ap()

    def mm(ap):
        return ap

    with tile.TileContext(nc) as tc:
        with (
            tc.tile_pool(name="const", bufs=1) as cpool,
            tc.tile_pool(name="h", bufs=1) as hpool,
            tc.tile_pool(name="o", bufs=2 if bf16 else 1) as opool,
            tc.tile_pool(name="w2s", bufs=4) as w2pool,
            tc.tile_pool(name="w3s", bufs=3) as w3pool,
            tc.tile_pool(name="ps", bufs=psum_bufs, space="PSUM") as pspool,
        ):
            # PE warm-up: the HAM clock gate releases (1.2 -> 2.4 GHz) after
            # ~3.4 us of sustained activity.  Run throwaway matmuls on scratch
            # SBUF while the first weight/activation DMAs are still in
            # flight, so the real matmuls start at full clock.
            scratch = cpool.tile([P, nck], DDT)
            nc.gpsimd.memset(scratch[:], 0.0)
            wps = pspool.tile([P, nck], dt.float32, tag="ps")
            for i in range(12):
                nc.tensor.matmul(
                    wps[:],
                    scratch[:, :P],
                    scratch[:],
                    start=(i == 0),
                    stop=(i == 11),
                )

            # DMA trigger instructions serialize at ~0.7 us apiece per engine
            # sequencer, and concurrent transfers share HBM bandwidth, so the
            # issue order is chosen to put the layer-1 critical path (W1
            # blocks + chunk-0 x) in front: W1 on the sync queue, biases and
            # x chunk 0 on the scalar queue, W3/output on gpsimd.
            xv = xT.rearrange("(ko p) b -> p ko b", p=P)
            x_sb = cpool.tile([P, K1T, b_l], DDT)
            for k in range(K1T):
                nc.sync.dma_start(x_sb[:, k, 0:nck], xv[:, k, 0:nck])
            w1_sb = cpool.tile([P, K1T, H1], DDT)
            for mi in range(H1T):
                nc.sync.dma_start(w1_sb[:, :, mi * P : (mi + 1) * P], w1t[mi])

            b1_sb = cpool.tile([P, H1T], dt.float32)
            nc.scalar.dma_start(b1_sb[:], b1[:])
            b2_sb = cpool.tile([P, H2T], dt.float32)
            nc.scalar.dma_start(b2_sb[:], b2[:])
            b3_sb = cpool.tile([P, DOT], dt.float32)
            nc.scalar.dma_start(b3_sb[:], b3[:])

            if bf16:
                w2_sb = cpool.tile([P, H1T, H2], DDT)
                for mi in range(H2T):
                    nc.sync.dma_start(w2_sb[:, :, mi * P : (mi + 1) * P], w2t[mi])

            ov = outT.rearrange("(mo p) b -> p mo b", p=P)

            for c in range(chunks):
                cs = slice(c * nck, (c + 1) * nck)
                if c + 1 < chunks:  # prefetch next chunk's x slice
                    ns = slice((c + 1) * nck, (c + 2) * nck)
                    for k in range(K1T):
                        nc.sync.dma_start(x_sb[:, k, ns], xv[:, k, ns])
                # layer 1: h1T = gelu(W1 @ xT + b1)
                h1_sb = hpool.tile([P, H1T, nck], DDT, tag="h1")
                for mi in range(H1T):
                    ps = pspool.tile([P, nck], dt.float32, tag="ps")
                    for k in range(K1T):
                        nc.tensor.matmul(
                            ps[:],
                            mm(w1_sb[:, k, mi * P : (mi + 1) * P]),
                            mm(x_sb[:, k, cs]),
                            start=(k == 0),
                            stop=(k == K1T - 1),
                        )
                    nc.scalar.activation(
                        h1_sb[:, mi], ps[:], act_fn, bias=b1_sb[:, mi : mi + 1]
                    )
                # layer 2: h2T = gelu(W2 @ h1T + b2)
                h2_sb = hpool.tile([P, H2T, nck], DDT, tag="h2")
                for mi in range(H2T):
                    if bf16:
                        wcol = w2_sb[:, :, mi * P : (mi + 1) * P]
                    else:
                        wcol = w2pool.tile([P, H1T, P], DDT, tag="w2col")
                        nc.sync.dma_start(wcol[:], w2t[mi])
                    ps = pspool.tile([P, nck], dt.float32, tag="ps")
                    for k in range(H1T):
                        nc.tensor.matmul(
                            ps[:],
                            mm(wcol[:, k]),
                            mm(h1_sb[:, k]),
                            start=(k == 0),
                            stop=(k == H1T - 1),
                        )
                    nc.scalar.activation(
                        h2_sb[:, mi], ps[:], act_fn, bias=b2_sb[:, mi : mi + 1]
                    )
                # layer 3: oT = W3 @ h2T + b3
                o_sb = opool.tile([P, DOT, nck], dt.float32, tag="o")
                for mi in range(DOT):
                    wcol = w3pool.tile([P, H2T, P], DDT, tag="w3col")
                    nc.gpsimd.dma_start(wcol[:], w3t[mi])
                    ps = pspool.tile([P, nck], dt.float32, tag="ps")
                    for k in range(H2T):
                        nc.tensor.matmul(
                            ps[:],
                            mm(wcol[:, k]),
                            mm(h2_sb[:, k]),
                            start=(k == 0),
                            stop=(k == H2T - 1),
                        )
                    nc.scalar.activation(
                        o_sb[:, mi], ps[:], act.Identity, bias=b3_sb[:, mi : mi + 1]
                    )
                    nc.sync.dma_start(ov[:, mi, cs], o_sb[:, mi])
    nc.compile()
    return nc


def _get_nc(variant):
    if variant not in _nc_cache:
        _nc_cache[variant] = _build_nc(variant)
    return _nc_cache[variant]


def _prep_inputs(variant, x, W1, b1, W2, b2, W3, b3):
    """Shard x along batch, pre-transpose weights, cast to the matmul dtype."""
    f32 = np.float32
    if variant == "bf16":
        def to(a):
            return np.ascontiguousarray(np.asarray(a, f32)).astype(ml_dtypes.bfloat16)
    else:
        def to(a):
            # Round to the float32r grid (11 mantissa bits, low 12 word bits
            # zero) -- same conversion walrus's fp32_to_fp32r performs.
            u = np.ascontiguousarray(np.asarray(a, f32)).view(np.uint32)
            return ((u + 0x800) & np.uint32(0xFFFFF000)).view(f32)

    # Pack per output-column block: wp[mi, p, ko, m] = W.T[ko*P+p, mi*P+m]
    # = W[mi*P+m, ko*P+p].
    def pack(W, O, I):
        Wr = np.asarray(W, f32).reshape(O // P, P, I // P, P)  # [mi,m,ko,p]
        return to(np.ascontiguousarray(Wr.transpose(0, 3, 2, 1)))  # [mi,p,ko,m]

    w1t = pack(W1, H1, D_IN)
    w2t = pack(W2, H2, H1)
    w3t = pack(W3, D_OUT, H2)
    def bpack(b, n):
        return np.ascontiguousarray(np.asarray(b, f32).reshape(n // P, P).T)

    b1f = bpack(b1, H1)
    b2f = bpack(b2, H2)
    b3f = bpack(b3, D_OUT)
    x = np.asarray(x, f32)
    in_maps = []
    for c in range(N_CORES):
        xs = x[c * B_L : (c + 1) * B_L]
        in_maps.append(
            {
                "xT": to(xs.T),
                "w1t": w1t,
                "w2t": w2t,
                "w3t": w3t,
                "b1": b1f,
                "b2": b2f,
                "b3": b3f,
            }
        )
    return in_maps


def kernel(x, W1, b1, W2, b2, W3, b3):
    global LAST_RESULT
    from concourse.bass_utils import run_bass_kernel_spmd

    variant = VARIANT
    nc = _get_nc(variant)
    in_maps = _prep_inputs(variant, x, W1, b1, W2, b2, W3, b3)
    res = run_bass_kernel_spmd(nc, in_maps, core_ids=list(range(N_CORES)))
    LAST_RESULT = res
    out = np.empty((B, D_OUT), np.float32)
    for c in range(N_CORES):
        out[c * B_L : (c + 1) * B_L] = np.asarray(res.results[c]["outT"]).T
    return out


# revision 52
# speedup vs baseline: 1.0042x; 1.0042x over previous
"""Fused 3-layer MLP (Linear+GELU, Linear+GELU, Linear) for Trainium2.

Strategy: 8-way data parallel over the batch dimension (16384 -> 2048 rows
per NeuronCore), weights replicated.  All operands are laid out on the host
so that the contraction dimension lands on SBUF partitions for both matmul
operands -- activations flow feature-partitioned ("transposed", [feat, batch])
through all three layers, so no on-device transposes are needed:

    h1T = gelu(W1 @ xT + b1)    lhsT = W1.T (pre-transposed on host)
    h2T = gelu(W2 @ h1T + b2)   lhsT = W2.T
    oT  =      W3 @ h2T + b3    lhsT = W3.T

Each layer's bias-add (+ GELU) is a single ScalarE activation op reading the
PSUM accumulator and writing SBUF.  The batch is processed in 4 chunks of 512
(one PSUM bank of fp32 per output tile).

Variants (MLP_VARIANT env var, default bf16):
  bf16: matmul operands rounded to bf16 (fp32 PSUM accumulate); W1/W2/x
        SBUF-resident, W3 streamed.  ~413 us HW, ~4.2e-3 l2 relative error.
  f32r: fp32 data rounded to the 11-mantissa-bit float32r grid; full PE rate
        at moving dim >= 512, but W2/W3 must be streamed and the 4-byte
        weight loads pace slower.  ~465 us HW, ~2.7e-4 l2 relative error.
"""

import os

import numpy as np
import ml_dtypes

# Problem shapes (fixed by the problem statement).
B, D_IN, H1, H2, D_OUT = 16384, 512, 2048, 2048, 1024
N_CORES = 8
B_L = B // N_CORES  # 2048 batch rows per core
P = 128
NCK = 512  # batch chunk = matmul moving free dim = one fp32 PSUM bank

VARIANT = os.environ.get("MLP_VARIANT", "bf16")  # "bf16" | "f32r"

LAST_RESULT = None  # BassKernelResults of the most recent run (for test.py)

_nc_cache = {}


def _build_nc(variant, b_l=B_L, nck=None, _act_name="Gelu"):
    import concourse.mybir as mybir
    import concourse.tile as tile
    from concourse import bacc

    dt = mybir.dt
    act = mybir.ActivationFunctionType
    act_fn = getattr(act, _act_name)
    bf16 = variant == "bf16"
    # float32r: fp32 with the mantissa rounded to 11 bits (low 12 bits of the
    # word zero).  The host pre-rounds all matmul inputs to that grid, so the
    # tensors are declared float32r end-to-end and the PE runs its
    # full-rate single-pass fp32r mode.
    DDT = dt.bfloat16 if bf16 else dt.float32r  # storage dtype
    if nck is None:
        # One fp32 PSUM bank per matmul output; a matmul may not cross the
        # bank boundary, so 512 is the widest legal moving dim.
        nck = 512
    chunks = b_l // nck
    psum_bufs = 6
    K1T, H1T, H2T, DOT = D_IN // P, H1 // P, H2 // P, D_OUT // P

    # Bacc (not raw Bass): its compile() pipeline legalizes multi-wait
    # instructions into event-semaphore pairs -- walrus accepts at most one
    # sync wait per instruction.
    nc = bacc.Bacc("TRN2", target_bir_lowering=False, debug=False)

    xT = nc.dram_tensor("xT", [D_IN, b_l], DDT, kind="ExternalInput").ap()
    # W1 is packed per output-column block (like the streamed weights below)
    # so layer 1 of chunk 0 can start after a single 256 KB block lands
    # instead of after the whole W1 load.
    w1t = nc.dram_tensor(
        "w1t", [H1 // P, P, D_IN // P, P], DDT, kind="ExternalInput"
    ).ap()
    # W2 is packed per output-column block on the host:
    # w2p[mi, p, ko, m] = W2T[ko*P+p, mi*P+m], so each [P, H1T, P] block DMA
    # reads contiguously per partition at full line rate, and blocks arrive
    # in the order layer 2 consumes them.
    w2t = nc.dram_tensor(
        "w2t", [H2 // P, P, H1 // P, P], DDT, kind="ExternalInput"
    ).ap()
    # W3 is always streamed (packed) -- it doesn't fit in SBUF alongside the
    # wide-chunk activations, and the DMA engines have plenty of headroom.
    w3t = nc.dram_tensor(
        "w3t", [D_OUT // P, P, H2 // P, P], DDT, kind="ExternalInput"
    ).ap()
    # Biases are host-transposed to [P, n_tiles] so the load is a plain
    # contiguous copy (the naive "(o p) -> p o" view is a 4-byte-element
    # gather whose descriptor generation alone costs ~2 us).
    b1 = nc.dram_tensor("b1", [P, H1 // P], dt.float32, kind="ExternalInput").ap()
    b2 = nc.dram_tensor("b2", [P, H2 // P], dt.float32, kind="ExternalInput").ap()
    b3 = nc.dram_tensor("b3", [P, D_OUT // P], dt.float32, kind="ExternalInput").ap()
    outT = nc.dram_tensor("outT", [D_OUT, b_l], dt.float32, kind="ExternalOutput").ap()

    def mm(ap):
        return ap

    with tile.TileContext(nc) as tc:
        with (
            tc.tile_pool(name="const", bufs=1) as cpool,
            tc.tile_pool(name="h", bufs=1) as hpool,
            tc.tile_pool(name="o", bufs=2 if bf16 else 1) as opool,
            tc.tile_pool(name="w2s", bufs=4) as w2pool,
            tc.tile_pool(name="w3s", bufs=3) as w3pool,
            tc.tile_pool(name="ps", bufs=psum_bufs, space="PSUM") as pspool,
        ):
            # PE warm-up: the HAM clock gate releases (1.2 -> 2.4 GHz) after
            # ~3.4 us of sustained activity.  Run throwaway matmuls on scratch
            # SBUF while the first weight/activation DMAs are still in
            # flight, so the real matmuls start at full clock.
            scratch = cpool.tile([P, nck], DDT)
            nc.vector.memset(scratch[:], 0.0)
            wps = pspool.tile([P, nck], dt.float32, tag="ps")
            for i in range(10):
                nc.tensor.matmul(
                    wps[:],
                    scratch[:, :P],
                    scratch[:],
                    start=(i == 0),
                    stop=(i == 9),
                )

            # DMA trigger instructions serialize at ~0.7 us apiece per engine
            # sequencer, and concurrent transfers share HBM bandwidth, so the
            # issue order is chosen to put the layer-1 critical path (W1
            # blocks + chunk-0 x) in front: W1 on the sync queue, biases and
            # x chunk 0 on the scalar queue, W3/output on gpsimd.
            xv = xT.rearrange("(ko p) b -> p ko b", p=P)
            x_sb = cpool.tile([P, K1T, b_l], DDT)
            for k in range(K1T):
                nc.sync.dma_start(x_sb[:, k, 0:nck], xv[:, k, 0:nck])
            w1_sb = cpool.tile([P, K1T, H1], DDT)
            for mi in range(H1T):
                nc.sync.dma_start(w1_sb[:, :, mi * P : (mi + 1) * P], w1t[mi])

            b1_sb = cpool.tile([P, H1T], dt.float32)
            nc.scalar.dma_start(b1_sb[:], b1[:])
            b2_sb = cpool.tile([P, H2T], dt.float32)
            nc.scalar.dma_start(b2_sb[:], b2[:])
            b3_sb = cpool.tile([P, DOT], dt.float32)
            nc.scalar.dma_start(b3_sb[:], b3[:])

            if bf16:
                w2_sb = cpool.tile([P, H1T, H2], DDT)
                for mi in range(H2T):
                    nc.sync.dma_start(w2_sb[:, :, mi * P : (mi + 1) * P], w2t[mi])

            ov = outT.rearrange("(mo p) b -> p mo b", p=P)

            for c in range(chunks):
                cs = slice(c * nck, (c + 1) * nck)
                if c + 1 < chunks:  # prefetch next chunk's x slice
                    ns = slice((c + 1) * nck, (c + 2) * nck)
                    for k in range(K1T):
                        nc.sync.dma_start(x_sb[:, k, ns], xv[:, k, ns])
                # layer 1: h1T = gelu(W1 @ xT + b1)
                h1_sb = hpool.tile([P, H1T, nck], DDT, tag="h1")
                for mi in range(H1T):
                    ps = pspool.tile([P, nck], dt.float32, tag="ps")
                    for k in range(K1T):
                        nc.tensor.matmul(
                            ps[:],
                            mm(w1_sb[:, k, mi * P : (mi + 1) * P]),
                            mm(x_sb[:, k, cs]),
                            start=(k == 0),
                            stop=(k == K1T - 1),
                        )
                    nc.scalar.activation(
                        h1_sb[:, mi], ps[:], act_fn, bias=b1_sb[:, mi : mi + 1]
                    )
                # layer 2: h2T = gelu(W2 @ h1T + b2)
                h2_sb = hpool.tile([P, H2T, nck], DDT, tag="h2")
                for mi in range(H2T):
                    if bf16:
                        wcol = w2_sb[:, :, mi * P : (mi + 1) * P]
                    else:
                        wcol = w2pool.tile([P, H1T, P], DDT, tag="w2col")
                        nc.sync.dma_start(wcol[:], w2t[mi])
                    ps = pspool.tile([P, nck], dt.float32, tag="ps")
                    for k in range(H1T):
                        nc.tensor.matmul(
                            ps[:],
                            mm(wcol[:, k]),
                            mm(h1_sb[:, k]),
                            start=(k == 0),
                            stop=(k == H1T - 1),
                        )
                    nc.scalar.activation(
                        h2_sb[:, mi], ps[:], act_fn, bias=b2_sb[:, mi : mi + 1]
                    )
                # layer 3: oT = W3 @ h2T + b3
                o_sb = opool.tile([P, DOT, nck], dt.float32, tag="o")
                for mi in range(DOT):
                    wcol = w3pool.tile([P, H2T, P], DDT, tag="w3col")
                    nc.gpsimd.dma_start(wcol[:], w3t[mi])
                    ps = pspool.tile([P, nck], dt.float32, tag="ps")
                    for k in range(H2T):
                        nc.tensor.matmul(
                            ps[:],
                            mm(wcol[:, k]),
                            mm(h2_sb[:, k]),
                            start=(k == 0),
                            stop=(k == H2T - 1),
                        )
                    nc.scalar.activation(
                        o_sb[:, mi], ps[:], act.Identity, bias=b3_sb[:, mi : mi + 1]
                    )
                    nc.sync.dma_start(ov[:, mi, cs], o_sb[:, mi])
    nc.compile()
    return nc


def _get_nc(variant):
    if variant not in _nc_cache:
        _nc_cache[variant] = _build_nc(variant)
    return _nc_cache[variant]


def _prep_inputs(variant, x, W1, b1, W2, b2, W3, b3):
    """Shard x along batch, pre-transpose weights, cast to the matmul dtype."""
    f32 = np.float32
    if variant == "bf16":
        def to(a):
            return np.ascontiguousarray(np.asarray(a, f32)).astype(ml_dtypes.bfloat16)
    else:
        def to(a):
            # Round to the float32r grid (11 mantissa bits, low 12 word bits
            # zero) -- same conversion walrus's fp32_to_fp32r performs.
            u = np.ascontiguousarray(np.asarray(a, f32)).view(np.uint32)
            return ((u + 0x800) & np.uint32(0xFFFFF000)).view(f32)

    # Pack per output-column block: wp[mi, p, ko, m] = W.T[ko*P+p, mi*P+m]
    # = W[mi*P+m, ko*P+p].
    def pack(W, O, I):
        Wr = np.asarray(W, f32).reshape(O // P, P, I // P, P)  # [mi,m,ko,p]
        return to(np.ascontiguousarray(Wr.transpose(0, 3, 2, 1)))  # [mi,p,ko,m]

    w1t = pack(W1, H1, D_IN)
    w2t = pack(W2, H2, H1)
    w3t = pack(W3, D_OUT, H2)
    def bpack(b, n):
        return np.ascontiguousarray(np.asarray(b, f32).reshape(n // P, P).T)

    b1f = bpack(b1, H1)
    b2f = bpack(b2, H2)
    b3f = bpack(b3, D_OUT)
    x = np.asarray(x, f32)
    in_maps = []
    for c in range(N_CORES):
        xs = x[c * B_L : (c + 1) * B_L]
        in_maps.append(
            {
                "xT": to(xs.T),
                "w1t": w1t,
                "w2t": w2t,
                "w3t": w3t,
                "b1": b1f,
                "b2": b2f,
                "b3": b3f,
            }
        )
    return in_maps


def kernel(x, W1, b1, W2, b2, W3, b3):
    global LAST_RESULT
    from concourse.bass_utils import run_bass_kernel_spmd

    variant = VARIANT
    nc = _get_nc(variant)
    in_maps = _prep_inputs(variant, x, W1, b1, W2, b2, W3, b3)
    res = run_bass_kernel_spmd(nc, in_maps, core_ids=list(range(N_CORES)))
    LAST_RESULT = res
    out = np.empty((B, D_OUT), np.float32)
    for c in range(N_CORES):
        out[c * B_L : (c + 1) * B_L] = np.asarray(res.results[c]["outT"]).T
    return out
